# revision 1
# baseline (speedup 1.0000x reference)
"""Sliding-window (tau=32) multi-head attention block with shared qkv projection,
distributed over 8 trn2 NeuronCores.

Sharding: data/sequence-parallel over the flattened (batch, token) axis —
8 shards of 1024 tokens. Each core receives its k/v slice with a 32-row
front halo (zeros at batch start), so projecting the concatenated buffer
reproduces the reference's pad-then-project semantics exactly (incl. bias).
"""

import numpy as np

import concourse.bacc as bacc
import concourse.bass as bass
import concourse.tile as tile
from concourse import mybir
from concourse.bass_utils import run_bass_kernel_spmd

B, N, E = 2, 4096, 256
H, TAU = 8, 32
HD = E // H
SCALING = HD**-0.5

NCORES = 8
T = B * N // NCORES  # 1024 q tokens per core
KT = T + 32  # kv rows incl. 32-row front halo
NB = KT // 32  # 33 kv blocks of 32
NQT = T // 128  # 8 q tiles of 128
NKTILE = (NB + 3) // 4  # 9 kpos tiles of (up to) 4 blocks

F32 = mybir.dt.float32
F32R = mybir.dt.float32r


def _host_constants():
    """Small constant tensors prepared host-side (shared by all cores)."""
    # band mask in S^T window coords: rows jj (kpos within block), cols ii
    # (q within the 64-wide window); valid iff ii - jj in [0, 31].
    jj = np.arange(32)[:, None]
    ii = np.arange(64)[None, :]
    band = ((ii - jj >= 0) & (ii - jj <= 31)).astype(np.float32)  # [32, 64]
    band128 = np.tile(band, (4, 1))  # [128, 64]
    normal = np.repeat(band128[:, None, :], H, axis=1)  # [128, H, 64]
    first = normal.copy()
    # kpos tile 0, block m=0 (partitions 0:32): left window half (q block -1)
    # does not exist.
    first[0:32, :, 0:32] = 0.0
    last = np.zeros_like(normal)
    # kpos tile 8 holds only block m=32 (partitions 0:32); only its left
    # window half (q block 31) exists.
    last[0:32, :, 0:32] = np.repeat(band[:, 0:32][:, None, :], H, axis=1)
    masks = np.stack([normal.reshape(128, H * 64),
                      first.reshape(128, H * 64),
                      last.reshape(128, H * 64)])  # [3, 128, 512]
    return masks.astype(np.float32)


def _prep_weights(W, b):
    WT = np.ascontiguousarray(W.T).astype(np.float32)  # [e_in, e_out]
    wT = WT.reshape(2, 128, 256).copy()
    # augmented V weights: per-head 33-wide column groups, ones col slot = 0
    WTaug = np.zeros((256, H * 33), np.float32)
    for h in range(H):
        WTaug[:, 33 * h : 33 * h + 32] = WT[:, 32 * h : 32 * h + 32]
    wTaug = WTaug.reshape(2, 128, H * 33).copy()
    b_aug = np.zeros((1, H * 33), np.float32)
    for h in range(H):
        b_aug[0, 33 * h : 33 * h + 32] = b[32 * h : 32 * h + 32]
        b_aug[0, 33 * h + 32] = 1.0
    b2 = b.reshape(2, 128).astype(np.float32)
    b2s = (SCALING * b2).astype(np.float32)
    return wT, wTaug, b_aug, b2, b2s


def build_program(stage=4, reps=1):
    # stage ordering: 1 < 2 < 31 (vpa) < 32 (scores mm) < 33 (exp) < 3 < 4
    s = {1: 1.0, 2: 2.0, 31: 3.1, 32: 3.2, 33: 3.3, 3: 3.5, 4: 4.0}[stage]
    nc = bacc.Bacc("TRN2", target_bir_lowering=False)

    q_d = nc.dram_tensor("q", [T, E], F32, kind="ExternalInput")
    k_d = nc.dram_tensor("k", [KT, E], F32, kind="ExternalInput")
    v_d = nc.dram_tensor("v", [KT, E], F32, kind="ExternalInput")
    wT_d = nc.dram_tensor("wT", [2, 128, 256], F32, kind="ExternalInput")
    wTaug_d = nc.dram_tensor("wTaug", [2, 128, H * 33], F32, kind="ExternalInput")
    baug_d = nc.dram_tensor("baug", [1, H * 33], F32, kind="ExternalInput")
    b2_d = nc.dram_tensor("b2", [2, 128], F32, kind="ExternalInput")
    b2s_d = nc.dram_tensor("b2s", [2, 128], F32, kind="ExternalInput")
    ident_d = nc.dram_tensor("ident", [128, 128], F32, kind="ExternalInput")
    ones_d = nc.dram_tensor("ones_row", [1, 128], F32, kind="ExternalInput")
    masks_d = nc.dram_tensor("masks", [3, 128, H * 64], F32, kind="ExternalInput")
    out_d = nc.dram_tensor("out", [T, E], F32, kind="ExternalOutput")

    # token chunking of the raw tensors (rows) into <=128-row chunks
    q_chunks = [(c * 128, 128) for c in range(NQT)]
    kv_chunks = [(c * 128, 128) for c in range(KT // 128)]
    if KT % 128:
        kv_chunks.append((KT - KT % 128, KT % 128))

    with tile.TileContext(nc) as tc:
        with (
            tc.tile_pool(name="consts", bufs=1) as consts,
            tc.tile_pool(name="raw", bufs=8) as raw_pool,
            tc.tile_pool(name="xT", bufs=1) as xT_pool,
            tc.tile_pool(name="proj", bufs=1) as proj_pool,
            tc.tile_pool(name="aw", bufs=1) as aw_pool,
            tc.tile_pool(name="ofin", bufs=4) as ofin_pool,
            tc.tile_pool(name="ps_proj", bufs=2, space="PSUM") as ps_proj,
            tc.tile_pool(name="ps_s", bufs=1, space="PSUM") as ps_s,
            tc.tile_pool(name="ps_o", bufs=1, space="PSUM") as ps_o,
        ):
            # ---- constants -------------------------------------------------
            ident = consts.tile([128, 128], F32)
            nc.sync.dma_start(out=ident, in_=ident_d.ap())
            ones_sb = consts.tile([1, 128], F32)
            nc.sync.dma_start(out=ones_sb, in_=ones_d.ap())
            ones_fr = consts.tile([1, 128], F32R)
            nc.vector.tensor_copy(ones_fr, ones_sb)
            baug_sb = consts.tile([1, H * 33], F32)
            nc.sync.dma_start(out=baug_sb, in_=baug_d.ap())
            baug_fr = consts.tile([1, H * 33], F32R)
            nc.vector.tensor_copy(baug_fr, baug_sb)
            b2_sb = consts.tile([128, 2], F32)
            b2s_sb = consts.tile([128, 2], F32)
            for o in range(2):
                nc.sync.dma_start(out=b2_sb[:, o : o + 1], in_=b2_d.ap()[o][:, None])
                nc.sync.dma_start(out=b2s_sb[:, o : o + 1], in_=b2s_d.ap()[o][:, None])
            wT_sb = consts.tile([128, 2, 256], F32)
            wTaug_sb = consts.tile([128, 2, H * 33], F32)
            for ki in range(2):
                nc.sync.dma_start(out=wT_sb[:, ki, :], in_=wT_d.ap()[ki])
                nc.sync.dma_start(out=wTaug_sb[:, ki, :], in_=wTaug_d.ap()[ki])
            wT_fr = consts.tile([128, 2, 256], F32R)
            wTaug_fr = consts.tile([128, 2, H * 33], F32R)
            nc.vector.tensor_copy(wT_fr, wT_sb)
            nc.vector.tensor_copy(wTaug_fr, wTaug_sb)
            masks_sb = consts.tile([128, 3, H * 64], F32)
            for i in range(3):
                nc.sync.dma_start(out=masks_sb[:, i, :], in_=masks_d.ap()[i])

            # ---- load raw + PE transpose -> xT (f32r) ----------------------
            for _rep in range(reps):
              _ = _rep  # noqa
              xT_q = xT_pool.tile([128, 2, T], F32R, tag="xTq", name="xT_q")
              xT_k = xT_pool.tile([128, 2, KT], F32R, tag="xTk")
              xT_v = xT_pool.tile([128, 2, KT], F32R, tag="xTv")

              def load_transpose(dram, xT, chunks, which):
                  pairs = [chunks[i : i + 2] for i in range(0, len(chunks), 2)]
                  for idx, pair in enumerate(pairs):
                      pt = ps_proj.tile([128, 512], F32, tag="psp", name="pt").rearrange(
                          "p (a b) -> p a b", a=2
                      )
                      base = pair[0][0]
                      tot = sum(pc for _, pc in pair)
                      for j, (c0, pc) in enumerate(pair):
                          rt = raw_pool.tile([128, 256], F32, tag="raw")
                          nc.sync.dma_start(
                              out=rt[:pc, :], in_=dram.ap()[c0 : c0 + pc]
                          )
                          for o in range(2):
                              nc.tensor.transpose(
                                  pt[:, o, 128 * j : 128 * j + pc],
                                  rt[:pc, 128 * o : 128 * o + 128],
                                  ident[:pc, :pc],
                              )
                      # drain both chunks + halves in one op; alternate engines
                      if idx % 2 == 0:
                          nc.scalar.activation(
                              xT[:, :, base : base + tot],
                              pt[:, :, :tot],
                              mybir.ActivationFunctionType.Copy,
                          )
                      else:
                          nc.vector.tensor_copy(
                              xT[:, :, base : base + tot], pt[:, :, :tot]
                          )

              def debug_dump(sb_ap, nelem):
                  dst = out_d.ap().rearrange("(p a) b -> p (a b)", p=128)
                  nc.sync.dma_start(out=dst[:, :nelem], in_=sb_ap)

              load_transpose(q_d, xT_q, q_chunks, "q")
              load_transpose(k_d, xT_k, kv_chunks, "k")
              load_transpose(v_d, xT_v, kv_chunks, "v")

              if s == 1:
                  debug_dump(xT_q.bitcast(F32), 2 * T)

              # ---- projections ----------------------------------------------
              qpT = proj_pool.tile([128, 2, T], F32, tag="qpT")
              kpT = proj_pool.tile([128, 2, KT], F32, tag="kpT")

              def project_T(xT, outT, tok_total, bias_sb):  # noqa: E306
                  if s < 2:
                      return
                  # outT[:, o, j] = sum_e_in wT[e_in, 128o + p] * xT[e_in, j] (+ bias)
                  j = 0
                  drain_idx = 0
                  while j < tok_total:
                      w = min(512, tok_total - j)
                      for o in range(2):
                          ps = ps_proj.tile([128, 512], F32, tag="psp")
                          for ki in range(2):
                              nc.tensor.matmul(
                                  ps[:, :w],
                                  wT_fr[:, ki, 128 * o : 128 * o + 128],
                                  xT[:, ki, j : j + w],
                                  start=(ki == 0),
                                  stop=(ki == 1),
                              )
                          scale = SCALING if bias_sb is b2s_sb else 1.0
                          if drain_idx % 2 == 0:
                              nc.scalar.activation(
                                  outT[:, o, j : j + w],
                                  ps[:, :w],
                                  mybir.ActivationFunctionType.Identity,
                                  bias=bias_sb[:, o : o + 1],
                                  scale=scale,
                              )
                          else:
                              nc.vector.tensor_scalar(
                                  outT[:, o, j : j + w],
                                  ps[:, :w],
                                  scale,
                                  bias_sb[:, o : o + 1],
                                  mybir.AluOpType.mult,
                                  mybir.AluOpType.add,
                              )
                          drain_idx += 1
                      j += w

              project_T(xT_q, qpT, T, b2_sb)
              project_T(xT_k, kpT, KT, b2s_sb)
              if s == 2:
                  debug_dump(qpT, 2 * T)

              # vp_aug natural layout, per 128-token chunk
              vpa = [
                  proj_pool.tile([128, H * 33], F32, tag=f"vpa{i}", name=f"vpa{i}")
                  for i in range(len(kv_chunks))
              ]
              for idx, (c0, pc) in enumerate(kv_chunks):
                  if s < 3.1:
                      break
                  ps = ps_proj.tile([128, 512], F32, tag="psp")
                  for ki in range(2):
                      nc.tensor.matmul(
                          ps[:pc, 0 : H * 33],
                          xT_v[:, ki, c0 : c0 + pc],
                          wTaug_fr[:, ki, :],
                          start=(ki == 0),
                          stop=False,
                      )
                  nc.tensor.matmul(
                      ps[:pc, 0 : H * 33],
                      ones_fr[:, :pc],
                      baug_fr,
                      start=False,
                      stop=True,
                  )
                  nc.vector.tensor_copy(vpa[idx][:pc, :], ps[:pc, 0 : H * 33])

              if s == 3.1:
                  debug_dump(vpa[0], H * 33)

              # ---- scores (S^T windowed) + exp + mask ------------------------
              # PSUM layout: [128 (sig,jj), 4 (hr -> bank), 128 (ht,64win)].
              # Same-col (sig) writers differing in row group hr land in
              # different banks (concurrent different-row matmuls to one
              # (col, bank) fault the device).
              aw = [
                  aw_pool.tile([128, 4, 128], F32, tag=f"aw{c}", name=f"aw{c}")
                  for c in range(NKTILE)
              ]
              # ---- out' matmuls + normalize ----------------------------------
              # No PSUM accumulation: the two kv-block contributions of each
              # q block go to separate banks (mi -> bank), summed on DVE.
              # Same-(col, bank) writers then always share a row group.
              def out_tile(t):
                  po = ps_o.tile([128, 2, H, 64], F32, tag="pso", name="po")
                  for gi in range(4):
                      g = 4 * t + gi
                      for h in range(H):
                          hr, ht = h % 4, h // 4
                          for mi, m in enumerate((g, g + 1)):
                              c, sig = m // 4, m % 4
                              half = 32 if m == g else 0
                              lhsT = aw[c][
                                  32 * sig : 32 * sig + 32, hr,
                                  64 * ht + half : 64 * ht + half + 32,
                              ]
                              rhs = vpa[c][
                                  32 * sig : 32 * sig + 32, 33 * h : 33 * h + 33
                              ]
                              nc.tensor.matmul(
                                  po[32 * gi : 32 * gi + 32, mi, h, 0:33],
                                  lhsT,
                                  rhs,
                                  start=True,
                                  stop=True,
                                  tile_position=(32 * sig, 32 * gi),
                              )
                  pb_sb = ofin_pool.tile([128, H, 33], F32, tag="pb_sb")
                  nc.scalar.activation(
                      pb_sb, po[:, 1, :, 0:33], mybir.ActivationFunctionType.Copy
                  )
                  osum = ofin_pool.tile([128, H, 33], F32, tag="osum")
                  nc.vector.scalar_tensor_tensor(
                      out=osum,
                      in0=po[:, 0, :, 0:33],
                      scalar=1.0,
                      in1=pb_sb,
                      op0=mybir.AluOpType.mult,
                      op1=mybir.AluOpType.add,
                  )
                  rec = ofin_pool.tile([128, H], F32, tag="rec")
                  nc.vector.reciprocal(rec, osum[:, :, 32])
                  ofin = ofin_pool.tile([128, H, 32], F32, tag="ofin")
                  rec_b = bass.AP(
                      tensor=rec.tensor,
                      offset=rec.offset,
                      ap=[rec.ap[0], [rec.ap[1][0], H], [0, 32]],
                  )
                  nc.vector.tensor_mul(ofin, osum[:, :, 0:32], rec_b)
                  nc.sync.dma_start(
                      out=out_d.ap()[128 * t : 128 * t + 128], in_=ofin
                  )

              for c in range(NKTILE):
                  if s < 3.2:
                      break
                  nsig = 4 if c < NKTILE - 1 else NB - 4 * c
                  ps = ps_s.tile([128, 4, 128], F32, tag="pss", padded_shape=[128, 4, 512])
                  if c == NKTILE - 1:
                      # only block m=32 is computed; clear the rest before exp
                      nc.vector.memset(ps[:, :, 0:128], 0.0)
                  for sig in range(nsig):
                      m = 4 * c + sig
                      for h in range(H):
                          hr, ht = h % 4, h // 4
                          lhsT = kpT[32 * hr : 32 * hr + 32, ht, 32 * m : 32 * m + 32]
                          if m == 0:
                              rhs = qpT[32 * hr : 32 * hr + 32, ht, 0:32]
                              outap = ps[32 * sig : 32 * sig + 32, hr,
                                         64 * ht + 32 : 64 * ht + 64]
                          elif m == NB - 1:
                              rhs = qpT[
                                  32 * hr : 32 * hr + 32, ht, 32 * (m - 1) : 32 * m
                              ]
                              outap = ps[32 * sig : 32 * sig + 32, hr,
                                         64 * ht : 64 * ht + 32]
                          else:
                              rhs = qpT[
                                  32 * hr : 32 * hr + 32,
                                  ht,
                                  32 * (m - 1) : 32 * (m + 1),
                              ]
                              outap = ps[32 * sig : 32 * sig + 32, hr,
                                         64 * ht : 64 * ht + 64]
                          nc.tensor.matmul(
                              outap,
                              lhsT,
                              rhs,
                              start=True,
                              stop=True,
                              tile_position=(32 * hr, 32 * sig),
                          )
                  # zero never-written PSUM regions so exp sees finite values
                  if c == 0:
                      nc.vector.memset(ps[0:32, :, 0:32], 0.0)
                      nc.vector.memset(ps[0:32, :, 64:96], 0.0)
                  if s < 3.3:
                      nc.vector.tensor_copy(aw[c], ps[:, :, 0:128])
                      continue
                  ex = aw_pool.tile([128, 4, 128], F32, tag="ex", bufs=3)
                  nc.scalar.activation(ex, ps[:, :, 0:128],
                                       mybir.ActivationFunctionType.Exp)
                  if s < 3.5:
                      nc.vector.tensor_copy(aw[c], ex)
                      continue
                  mi = 0 if 0 < c < NKTILE - 1 else (1 if c == 0 else 2)
                  nc.vector.tensor_mul(aw[c], ex, masks_sb[:, mi, :].rearrange(
                      "p (r w) -> p r w", r=4))
                  if s >= 4 and c >= 1:
                      out_tile(c - 1)

              if 3.2 <= s < 4:
                  debug_dump(aw[0].rearrange("p r w -> p (r w)"), 4 * 128)


    nc.compile()
    return nc


_NC_CACHE = None


def _get_nc():
    global _NC_CACHE
    if _NC_CACHE is None:
        _NC_CACHE = build_program()
    return _NC_CACHE


def make_in_maps(query, key, value, W, b):
    query = np.asarray(query, np.float32)
    key = np.asarray(key, np.float32)
    value = np.asarray(value, np.float32)
    W = np.asarray(W, np.float32)
    b = np.asarray(b, np.float32)

    wT, wTaug, b_aug, b2, b2s = _prep_weights(W, b)
    masks = _host_constants()
    ident = np.eye(128, dtype=np.float32)
    ones_row = np.ones((1, 128), np.float32)

    qf = query.reshape(B * N, E)
    kf = key.reshape(B * N, E)
    vf = value.reshape(B * N, E)
    shards_per_b = NCORES // B
    in_maps = []
    for c in range(NCORES):
        s0 = c * T
        halo0 = s0 - 32
        if c % shards_per_b == 0:
            halo_k = np.zeros((32, E), np.float32)
            halo_v = np.zeros((32, E), np.float32)
        else:
            halo_k = kf[halo0:s0]
            halo_v = vf[halo0:s0]
        in_maps.append(
            {
                "q": np.ascontiguousarray(qf[s0 : s0 + T]),
                "k": np.ascontiguousarray(np.concatenate([halo_k, kf[s0 : s0 + T]])),
                "v": np.ascontiguousarray(np.concatenate([halo_v, vf[s0 : s0 + T]])),
                "wT": wT,
                "wTaug": wTaug,
                "baug": b_aug,
                "b2": b2,
                "b2s": b2s,
                "ident": ident,
                "ones_row": ones_row,
                "masks": masks,
            }
        )
    return in_maps


def kernel(query, key, value, W, b):
    nc = _get_nc()
    in_maps = make_in_maps(query, key, value, W, b)
    res = run_bass_kernel_spmd(nc, in_maps, list(range(NCORES)))
    out = np.concatenate([res.results[c]["out"] for c in range(NCORES)], axis=0)
    return out.reshape(B, N, E).astype(np.float32)

